# revision 38
# baseline (speedup 1.0000x reference)
"""GPT causal attention block (B=2, S=2048, H=16, hd=64, d=1024),
sharded over 8 NeuronCores as (batch x head-group): core c -> batch c//4,
heads 4*(c%4) .. 4*(c%4)+3.  Host converts bf16 partials to fp32 and sums
the 4 row-parallel partials per batch.

All matmuls run in bf16 (1 PE cycle/row vs fp32's 4) with fp32 PSUM
accumulation; rel err ~3.5e-3 vs the fp32 reference (gate 2e-2).

Per-core device program (per 512-token chunk t, fully software-pipelined):
  qkT chunk = Wqk_shard.T @ xT   [512, 512]x4  (q cols of W pre-scaled 1/8)
  v chunk   = x @ Wv_shard       stored ones-augmented [128, kb, h, 65]
  attention for q-chunk t, per head, per k-block pair (j0, j1):
      ST2 = kT_h.T-contract qT_h   2x[128 ktok, 512 qtok]  (K=64)
      PT2 = exp(ST2)  one Act call per pair (amortizes the 222-cycle
            SBUF access); tril mask on diagonal blocks via DVE
      O  += v_aug_j.T @ PT_j       [65, 512]; row 64 = softmax sums l
  attT = O[0:64] * (1/l)  broadcast via DRAM-scratch bounce, emission
         deferred a few pairs so the 2-DMA latency never blocks anything
         (final head of the last chunk uses an on-chip ones-matmul
         broadcast instead: nothing is left to hide DMA latency behind)
  out chunk = attT.T @ Wo_shard (+ bo on group leader)  [512, 1024] bf16

Emission interleaves work units at sub-chunk granularity so the in-order
engines never starve each other: the exp stream (Activation engine,
~76us total) runs concurrently with the PE's projection matmuls --
phase1(t+1) groups and phase3(<=t-1) tiles are sprinkled between
attention pairs, and each pair's PV matmuls are emitted one pair late
(lag-1) to stay out of the exp's shadow.  TimelineSim: ~135us/core
(PE busy ~116us = 86% occupancy) vs ~500us for the fp32 baseline.
"""
import sys
import numpy as np

sys.path.insert(0, "/opt/trn_rl_repo")

import concourse.bass as bass
import concourse.mybir as mybir
import concourse.tile as tile

B, S, D, NH, HD = 2, 2048, 1024, 16, 64
HPC = 4            # heads per core
NKB = S // 128     # 16 k-blocks
NQC = S // 512     # 4 q-chunks
F32 = mybir.dt.float32
BF16 = mybir.dt.bfloat16
MAX_WAITS = 1      # one sync-wait per NoOp; walrus limits are per-engine and tight


def _split_excess_waits(nc, max_waits=MAX_WAITS):
    """walrus CoreV3 rejects instructions with more than ~4 sync waits; move
    the excess onto same-engine NoOps inserted just before the instruction."""
    n_split = 0
    for blk in nc.m.functions[0].blocks:
        for idx in range(len(blk.instructions) - 1, -1, -1):
            inst = blk.instructions[idx]
            if isinstance(inst, mybir.InstISA) and inst.isa_opcode == 176:
                # EVENT_SEMAPHORE_RANGE_CLEAR mis-encodes for this walrus
                # ("ISA wrong length"); sems are re-zeroed by NRT per load.
                blk.instructions.pop(idx)
        idx = 0
        while idx < len(blk.instructions):
            inst = blk.instructions[idx]
            si = inst.sync_info
            lim = 0 if isinstance(inst, mybir.InstMatmult) else max_waits
            if si is not None and si.on_wait and len(si.on_wait) > lim:
                waits = list(si.on_wait)
                si.on_wait = waits[len(waits) - lim:] if lim else []
                rest = waits[:len(waits) - lim] if lim else waits
                for i in range(0, len(rest), max_waits):
                    nop = mybir.InstNoOp(
                        name=nc.get_next_instruction_name(),
                        sync_info=mybir.SyncInfo(
                            on_wait=rest[i:i + max_waits], on_update=[]
                        ),
                        bass_nofuse=True,
                        engine=inst.engine,
                    )
                    nc.register_instruction(nop)
                    blk.instructions.insert(idx, nop)
                    idx += 1
                n_split += 1
            idx += 1
    return n_split


def _build():
    nc = bass.Bass("TRN2", target_bir_lowering=False, debug=False, num_devices=8)
    xT = nc.declare_dram_parameter("xT", [D, S], BF16, isOutput=False)
    wqk = nc.declare_dram_parameter("wqk", [D, 512], BF16, isOutput=False)
    wv = nc.declare_dram_parameter("wv", [D, 256], BF16, isOutput=False)
    bqk = nc.declare_dram_parameter("bqk", [512], F32, isOutput=False)
    bv = nc.declare_dram_parameter("bv", [256], F32, isOutput=False)
    wo = nc.declare_dram_parameter("wo", [256, D], BF16, isOutput=False)
    bo = nc.declare_dram_parameter("bo", [D], F32, isOutput=False)
    out = nc.declare_dram_parameter("out", [S, D], BF16, isOutput=True)
    lscr = nc.dram_tensor("lscr", [NQC, HPC, 512], F32)

    with tile.TileContext(nc) as tc:
        with (
            tc.tile_pool(name="singles", bufs=1) as singles,
            tc.tile_pool(name="xtp", bufs=3) as xtp,
            tc.tile_pool(name="pt", bufs=8) as ptp,
            tc.tile_pool(name="zs", bufs=4) as zsp,
            tc.tile_pool(name="stp", bufs=2, space="PSUM") as stp,
            tc.tile_pool(name="otp", bufs=2, space="PSUM") as otp,
            tc.tile_pool(name="mix", bufs=2, space="PSUM") as mix,
        ):
            # ---- resident SBUF tensors ----
            wqk_sb = singles.tile([128, 8, 512], BF16)     # [dblk] x 512 qk cols
            wv_sb = singles.tile([128, 8, 256], BF16)
            wo_sb = singles.tile([128, 2, D], BF16)        # 2 feat blocks
            qT_sb = singles.tile([128, 2, S], BF16)        # q, heads pair-stacked
            kT_sb = singles.tile([128, 2, S], BF16)
            v_sb = singles.tile([128, NKB, HPC, 65], BF16)  # ones-augmented v
            attT_sb = singles.tile([128, 2, S], BF16)      # unnormed attn out^T
            bqk_sb = singles.tile([128, 4], F32)           # per-feat-block bias col
            bv_sb = singles.tile([128, 256], F32)          # bv partition-bcast
            bo_sb = singles.tile([128, D], F32)            # bo partition-bcast
            tril_sb = singles.tile([128, 128], BF16)       # keep iff qt >= kt
            ones_sb = singles.tile([65, 64], BF16)         # final 1/l bcast
            # (row 64: matmul needs lhsT/rhs at the same base partition)

            # startup DMAs ordered so the first qk matmul group's inputs
            # (wqk, bqk, x chunk 0) land first on the serial DMA queue
            xt0 = xtp.tile([128, 8, 512], BF16, tag="xt")
            for q in range(4):
                rows = slice(q * 256, (q + 1) * 256)
                dblk = slice(q * 2, (q + 1) * 2)
                nc.sync.dma_start(
                    out=wqk_sb[:, dblk, :],
                    in_=wqk[rows, :].rearrange("(db p) c -> p db c", p=128),
                )
                nc.sync.dma_start(
                    out=xt0[:, dblk, :],
                    in_=xT[rows, 0:512].rearrange("(db p) c -> p db c", p=128),
                )
                if q == 0:
                    nc.sync.dma_start(
                        out=bqk_sb,
                        in_=bqk[:].rearrange("(blk p) -> p blk", p=128),
                    )
            nc.sync.dma_start(
                out=wv_sb, in_=wv[:, :].rearrange("(db p) c -> p db c", p=128)
            )
            nc.sync.dma_start(
                out=bv_sb,
                in_=bass.AP(tensor=bv[:].tensor, offset=bv[:].offset, ap=[[0, 128], [1, 256]]),
            )
            nc.sync.dma_start(
                out=wo_sb, in_=wo[:, :].rearrange("(fb p) c -> p fb c", p=128)
            )
            nc.sync.dma_start(
                out=bo_sb,
                in_=bass.AP(tensor=bo[:].tensor, offset=bo[:].offset, ap=[[0, 128], [1, D]]),
            )
            nc.vector.memset(v_sb[:, :, :, 64:65], 1.0)
            nc.vector.memset(ones_sb[64:65, :], 1.0)
            # tril_sb[kt, qt] = 1.0 if qt >= kt else 0 (upper-tri incl diag)
            nc.gpsimd.memset(tril_sb, 0.0)
            nc.gpsimd.affine_select(
                out=tril_sb, in_=tril_sb,
                compare_op=mybir.AluOpType.is_gt,
                fill=1.0, base=0, pattern=[[-1, 128]], channel_multiplier=1,
            )

            def phase3(c4):
                # out projection for token chunk c4 (emitted one chunk late
                # so its attT deps are long satisfied when the in-order PE
                # reaches it)
                for tb in range(4 * c4, 4 * c4 + 4):
                    zs = zsp.tile([128, 1024], BF16)
                    for oc in range(2):
                        ps = mix.tile([128, 512], F32, tag="mx")
                        for fb in range(2):
                            nc.tensor.matmul(
                                ps, attT_sb[:, fb, tb * 128:(tb + 1) * 128],
                                wo_sb[:, fb, oc * 512:(oc + 1) * 512],
                                start=(fb == 0), stop=(fb == 1),
                            )
                        nc.vector.tensor_add(
                            zs[:, oc * 512:(oc + 1) * 512], ps,
                            bo_sb[:, oc * 512:(oc + 1) * 512],
                        )
                    nc.sync.dma_start(
                        out=out[tb * 128:(tb + 1) * 128, :], in_=zs
                    )

            xts = {0: xt0}
            pending = []  # (h, q_lo, ot, lt) normalizations not yet emitted
            final_pending = []  # last head: normalized via on-chip bcast

            def flush_pending():
                # normalize head h: 1/l row (partition 64 of ot) was
                # reciprocal'd into lt; bounce it through a DRAM scratch row
                # to broadcast onto the head's 64 partitions, then scale O
                # while moving into attT. Deferred a few ST pairs (or into
                # the next chunk's stream) so nothing waits on the 2-DMA
                # latency.
                for h, ql, ot, lt in pending:
                    hp, c4f = h // 2, ql // 512
                    nc.sync.dma_start(out=lscr[c4f, h, :], in_=lt[64:65, :])
                    rbc = ptp.tile([64, 512], F32, tag="rbc")
                    lap = lscr[c4f, h, :]
                    nc.sync.dma_start(
                        out=rbc,
                        in_=bass.AP(tensor=lap.tensor, offset=lap.offset,
                                    ap=[[0, 64], [1, 512]]),
                    )
                    if h % 2 == 0:
                        nc.vector.tensor_mul(
                            attT_sb[0:64, hp, ql:ql + 512], ot[0:64, :], rbc
                        )
                    else:
                        stg = ptp.tile([64, 512], BF16, tag="stg")
                        nc.vector.tensor_mul(stg, ot[0:64, :], rbc)
                        nc.sync.dma_start(
                            out=attT_sb[64:128, hp, ql:ql + 512], in_=stg
                        )
                pending.clear()

            def qk_group(t, fb):  # fb 0,1: q; 2,3: k
                xt = xts[t]
                ps = mix.tile([128, 512], F32, tag="mx")
                for d in range(8):
                    nc.tensor.matmul(
                        ps, wqk_sb[:, d, fb * 128:(fb + 1) * 128], xt[:, d, :],
                        start=(d == 0), stop=(d == 7),
                    )
                dst = (qT_sb if fb < 2 else kT_sb)[:, fb % 2, t * 512:(t + 1) * 512]
                nc.vector.tensor_scalar_add(dst, ps, bqk_sb[:, fb:fb + 1])

            def v_group(t, tb):
                xt = xts[t]
                psv = mix.tile([128, 512], F32, tag="mx")
                for d in range(8):
                    nc.tensor.matmul(
                        psv[:, 0:256], xt[:, d, tb * 128:(tb + 1) * 128],
                        wv_sb[:, d, :],
                        start=(d == 0), stop=(d == 7),
                    )
                nc.vector.tensor_add(
                    out=v_sb[:, t * 4 + tb, :, 0:64],
                    in0=psv[:, 0:256].rearrange("p (h e) -> p h e", h=4),
                    in1=bv_sb.rearrange("p (h e) -> p h e", h=4),
                )

            def ph3_unit(tb):
                zs = zsp.tile([128, 1024], BF16)
                for oc in range(2):
                    ps = mix.tile([128, 512], F32, tag="mx")
                    for fb in range(2):
                        nc.tensor.matmul(
                            ps, attT_sb[:, fb, tb * 128:(tb + 1) * 128],
                            wo_sb[:, fb, oc * 512:(oc + 1) * 512],
                            start=(fb == 0), stop=(fb == 1),
                        )
                    nc.vector.tensor_add(
                        zs[:, oc * 512:(oc + 1) * 512], ps,
                        bo_sb[:, oc * 512:(oc + 1) * 512],
                    )
                    nc.sync.dma_start(
                        out=out[tb * 128:(tb + 1) * 128, oc * 512:(oc + 1) * 512],
                        in_=zs[:, oc * 512:(oc + 1) * 512],
                    )

            def ph1_units(t):
                from functools import partial
                return [partial(qk_group, t, fb) for fb in range(4)] + \
                       [partial(v_group, t, tb) for tb in range(4)]

            def ph3_units(c4):
                from functools import partial
                return [partial(ph3_unit, tb) for tb in range(4 * c4, 4 * c4 + 4)]

            # phase 1 of chunk 0 runs un-interleaved (nothing to overlap yet)
            for u in ph1_units(0):
                u()

            for t in range(NQC):
                c4 = t
                q_lo = c4 * 512
                njb = 4 * c4 + 4
                npairs = njb // 2

                # filler PE work sprinkled between this chunk's attention
                # pairs: next chunk's projections + previous chunk's output
                # projection (the latter only after the previous chunk's last
                # normalize has been flushed)
                fillers = list(ph1_units(t + 1)) if t + 1 < NQC else []
                if t == NQC - 1:
                    late = ph3_units(NQC - 3) + ph3_units(NQC - 2)
                elif t >= 2:
                    late = ph3_units(t - 2)
                else:
                    late = []
                if t + 1 < NQC:
                    xtn = xtp.tile([128, 8, 512], BF16, tag="xt")
                    nc.sync.dma_start(
                        out=xtn,
                        in_=xT[:, (t + 1) * 512:(t + 2) * 512].rearrange(
                            "(db p) c -> p db c", p=128
                        ),
                    )
                    xts[t + 1] = xtn

                total_pairs = npairs * HPC
                # hold fillers for the first few pairs (xt prefetch in flight)
                warmup = 5
                acc = 0.0
                pair_no = 0
                flushed = False
                flush_at = min(2, npairs - 1)

                def draw_fillers():
                    # adaptive rate; late (ph3) units additionally held until
                    # a third of the way in, where exp-wait gaps concentrate
                    nonlocal acc
                    late_ok = flushed and pair_no >= total_pairs // 2
                    pool = fillers + (late if late_ok else [])
                    if pair_no < warmup or not pool:
                        return
                    acc += len(pool) / max(1, total_pairs - pair_no)
                    while acc >= 1.0 and (fillers or (late_ok and late)):
                        acc -= 1.0
                        u = fillers.pop(0) if fillers else late.pop(0)
                        u()

                for h in (1, 3, 0, 2):  # evens last: their normalize chain
                    hp, hb = h // 2, (h % 2) * 64  # skips the stg DMA hop
                    ot = otp.tile([128, 512], F32, tag="ot")

                    def tril_pv(pairs):
                        # masks first so the DVE is a full PV ahead of the PE
                        for pt2, j, lo in pairs:
                            if j - 4 * c4 >= 0:  # diagonal block: causal mask
                                nc.vector.tensor_mul(
                                    pt2[:, lo:lo + 128], pt2[:, lo:lo + 128],
                                    tril_sb,
                                )
                        for pt2, j, lo in pairs:
                            nc.tensor.matmul(
                                ot[0:65, lo:512], v_sb[:, j, h, :],
                                pt2[:, lo:512],
                                start=(j == 0), stop=(j == njb - 1),
                            )

                    defer = None
                    for i in range(npairs):
                        j0, j1 = 2 * i, 2 * i + 1
                        lo0 = max(j0 - 4 * c4, 0) * 128
                        lo1 = max(j1 - 4 * c4, 0) * 128
                        st2 = stp.tile([128, 2, 512], F32, tag="st")
                        pt2 = ptp.tile([128, 2, 512], BF16, tag="pt")
                        for jj, (j, lo) in enumerate(((j0, lo0), (j1, lo1))):
                            nc.tensor.matmul(
                                st2[:, jj, lo:512],
                                kT_sb[hb:hb + 64, hp, j * 128:(j + 1) * 128],
                                qT_sb[hb:hb + 64, hp, q_lo + lo:q_lo + 512],
                                start=True, stop=True,
                            )
                        # one exp per block pair; cols [lo0, lo1) of the second
                        # slice hold stale PSUM (finite), exp'd but never read
                        nc.scalar.activation(
                            out=pt2[:, :, lo0:512], in_=st2[:, :, lo0:512],
                            func=mybir.ActivationFunctionType.Exp,
                        )
                        # lag-1: this pair's PV matmuls are emitted after the
                        # NEXT pair's ST+exp so the in-order PE doesn't sit in
                        # the exp's shadow; fillers slot into the same gap
                        if defer is not None:
                            tril_pv(defer)
                        defer = [(pt2[:, 0, :], j0, lo0), (pt2[:, 1, :], j1, lo1)]
                        if i == flush_at and pending:
                            flush_pending()
                            flushed = True
                        pair_no += 1
                        draw_fillers()
                    tril_pv(defer)
                    if t == NQC - 1 and h == 2:
                        # final head: its normalize is tail-exposed, so use
                        # the on-chip path (bf16 bcast matmul) instead of the
                        # ~6us DRAM round-trip; bf16 1/l costs ~0.4% on this
                        # head only
                        lt = ptp.tile([65, 512], BF16, tag="ltb")
                        with nc.allow_low_precision(reason="final-head 1/l bf16"):
                            nc.vector.reciprocal(lt[64:65, :], ot[64:65, :])
                        final_pending.append((h, q_lo, ot, lt))
                    else:
                        lt = ptp.tile([65, 512], F32, tag="lt")
                        nc.vector.reciprocal(lt[64:65, :], ot[64:65, :])
                        pending.append((h, q_lo, ot, lt))

                # any fillers not yet drawn (short chunks)
                for u in fillers:
                    u()
                if flushed:
                    for u in late:
                        u()
                else:
                    late_leftover = late
                    if late_leftover:
                        flush_pending()
                        for u in late_leftover:
                            u()

            flush_pending()
            for h, ql, ot, lt in final_pending:
                bc = mix.tile([128, 512], F32, tag="mx")
                nc.tensor.matmul(
                    bc[0:64, :], ones_sb[64:65, :], lt[64:65, :],
                    start=True, stop=True,
                )
                rbc = ptp.tile([64, 512], F32, tag="rbc")
                nc.scalar.activation(
                    out=rbc, in_=bc[0:64, :],
                    func=mybir.ActivationFunctionType.Identity,
                )
                nc.vector.tensor_mul(
                    attT_sb[0:64, h // 2, ql:ql + 512], ot[0:64, :], rbc
                )
            for u in ph3_units(NQC - 1):
                u()
    _split_excess_waits(nc)
    return nc


_NC = None


def _get_nc():
    global _NC
    if _NC is None:
        _NC = _build()
    return _NC


def make_in_maps(x, Wqkv, bqkv, Wo, bo):
    import ml_dtypes
    bf16 = ml_dtypes.bfloat16
    x = np.asarray(x, np.float32)
    Wqkv = np.asarray(Wqkv, np.float32)
    bqkv = np.asarray(bqkv, np.float32)
    Wo = np.asarray(Wo, np.float32)
    bo = np.asarray(bo, np.float32)
    zeros_bo = np.zeros_like(bo)
    in_maps = []
    for c in range(8):
        b, g = c // 4, c % 4
        cs = slice(g * 4 * HD, (g + 1) * 4 * HD)  # 256 head cols
        wq = Wqkv[:, 0:D][:, cs] * 0.125  # fold 1/sqrt(hd) into q
        wk = Wqkv[:, D:2 * D][:, cs]
        wv = Wqkv[:, 2 * D:3 * D][:, cs]
        bq = bqkv[0:D][cs] * 0.125
        bk = bqkv[D:2 * D][cs]
        bv = bqkv[2 * D:3 * D][cs]
        in_maps.append({
            "xT": np.ascontiguousarray(x[b].T).astype(bf16),
            "wqk": np.ascontiguousarray(np.concatenate([wq, wk], axis=1)).astype(bf16),
            "wv": np.ascontiguousarray(wv).astype(bf16),
            "bqk": np.ascontiguousarray(np.concatenate([bq, bk])),
            "bv": np.ascontiguousarray(bv),
            "wo": np.ascontiguousarray(Wo[cs, :]).astype(bf16),
            "bo": bo if g == 0 else zeros_bo,
        })
    return in_maps


def run_spmd(in_maps, trace=False):
    from concourse.bass_utils import run_bass_kernel_spmd
    return run_bass_kernel_spmd(_get_nc(), in_maps, list(range(8)), trace=trace)


def kernel(x, mask, Wqkv, bqkv, Wo, bo):
    """Full inputs in, full output out. mask is always causal-tril; causality
    is implemented structurally on device."""
    res = run_spmd(make_in_maps(x, Wqkv, bqkv, Wo, bo))
    outs = [np.asarray(res.results[c]["out"], dtype=np.float32) for c in range(8)]
    full = np.empty((B, S, D), np.float32)
    for b in range(B):
        full[b] = outs[4 * b + 0] + outs[4 * b + 1] + outs[4 * b + 2] + outs[4 * b + 3]
    return full
